# revision 33
# baseline (speedup 1.0000x reference)
# MoE block (top-2 of 8 experts) on 8 trn2 NeuronCores, expert-parallel.
#
# Sharding strategy:
#   - Core e owns expert e's weights (expert-parallel: each weight byte is read
#     from HBM exactly once across the fleet).
#   - Routing (x @ w_router.T, top-2, softmax) + token dispatch are computed on
#     the host as part of input sharding; core e receives the (transposed,
#     padded) batch of tokens routed to expert e.
#   - Device kernel per core: h.T = gelu(w_up @ x_g.T + b_up);
#     y.T = w_down @ h.T + b_down  — features on partitions, tokens on the
#     matmul free dimension, so every DMA is contiguous (no on-device
#     transposes needed).
#   - Unshard: host scatter-adds the per-expert outputs weighted by the top-2
#     softmax router weights.
import os
import time

import numpy as np

B, S, D, U, E, TOPK = 2, 2048, 1024, 4096, 8, 2
T = B * S
P = 128

_last_results = None  # BassKernelResults of the most recent device run (for test.py)
_prog_cache = {}


def _split_blocks(C):
    """Split C token columns into blocks of <=512 (PSUM bank / fp32 matmul
    free-dim limit), as equal as possible in multiples of 128."""
    nb = -(-C // 512)
    base = C // (128 * nb) * 128
    blocks = [base] * nb
    rem = C - base * nb
    i = 0
    while rem > 0:
        add = min(128, rem)
        blocks[i % nb] += add
        rem -= add
        i += 1
    assert sum(blocks) == C and all(b <= 512 for b in blocks)
    return blocks


def _mm_dtype_name():
    # fp16: same PE rate as bf16 (1 cyc/row) but 11-bit mantissa -> ~4e-4
    # relative error vs the fp32 reference (values here are far inside fp16
    # range). Measured: fp32 1017us/1.8e-6, fp32r 458us/2.1e-4,
    # bf16 357us/3.3e-3, fp16 346us/4.1e-4.
    return os.environ.get("KERNEL_MM_DTYPE", "fp16")


def _build_program(C):
    import concourse.bacc as bacc
    import concourse.mybir as mybir
    import concourse.tile as tile

    # Matmul operand dtype (measured issue rates on TRN2): fp32 ~4 cyc/row
    # (lowered to 2 half-rate passes), fp32r ~1.5, bf16/fp16 ~1 cyc/row.
    # PSUM accumulation is fp32 throughout.
    dt = {
        "fp32": mybir.dt.float32,
        "fp32r": mybir.dt.float32r,
        "bf16": mybir.dt.bfloat16,
        "fp16": mybir.dt.float16,
    }[_mm_dtype_name()]
    dt_bias = mybir.dt.float32
    dt_out = mybir.dt.float32
    KU = D // P  # 8   k-subtiles for the up-projection (contract over D)
    NU = U // P  # 32  output tiles of the up-projection
    KD = U // P  # 32  k-subtiles for the down-projection (contract over U)
    ND = D // P  # 8   output tiles of the down-projection

    nc = bacc.Bacc("TRN2", target_bir_lowering=False, debug=False, num_devices=E)

    xgT = nc.dram_tensor("xgT", [D, C], dt, kind="ExternalInput")  # gathered x, transposed
    wuT = nc.dram_tensor("wuT", [D, U], dt, kind="ExternalInput")  # w_up[e].T
    wdT = nc.dram_tensor("wdT", [U, D], dt, kind="ExternalInput")  # w_down[e].T
    bu = nc.dram_tensor("bu", [P, NU], dt_bias, kind="ExternalInput")  # b_up[e] as [128, 32]
    bd = nc.dram_tensor("bd", [P, ND], dt_bias, kind="ExternalInput")  # b_down[e] as [128, 8]
    yT = nc.dram_tensor("yT", [D, C], dt_out, kind="ExternalOutput")

    xg3 = xgT.ap().rearrange("(ko p) c -> p ko c", p=P)  # [128, 8, C]
    wu3 = wuT.ap().rearrange("(ko p) u -> p ko u", p=P)  # [128, 8, U]
    wd3 = wdT.ap().rearrange("(ko p) d -> p ko d", p=P)  # [128, 32, D]
    y3 = yT.ap().rearrange("(ko p) c -> p ko c", p=P)  # [128, 8, C]

    with tile.TileContext(nc) as tc:
        with (
            tc.tile_pool(name="const", bufs=1) as const,
            tc.tile_pool(name="weights", bufs=1) as wpool,
            tc.tile_pool(name="xpool", bufs=1) as xpool,
            tc.tile_pool(name="hpool", bufs=NU + 3) as hpool,
            tc.tile_pool(name="ypool", bufs=3) as ypool,
            tc.tile_pool(name="psum", bufs=8, space="PSUM") as psum_pool,
        ):
            blocks = _split_blocks(C)

            # DMA emission order tracks first-use order: x block 0, then w_up
            # (first up-chains), then remaining x blocks, biases, w_down.
            # Early transfers are enqueued on BOTH HWDGE-capable engines
            # (Scalar + Sync, ~0.7us per enqueue each) so enqueue
            # serialization doesn't pace the startup ramp. Scalar's 9
            # enqueues finish ~14us, well before its first gelu (~20us).
            xbs = [None] * len(blocks)
            xbs[0] = xpool.tile([P, KU, blocks[0]], dt, tag="x0", name="xb0")

            # Both weight matrices are SBUF-resident (16.8 MB in fp16): each
            # is loaded exactly once, as per-k-subtile fully-contiguous DMAs
            # that spread across the DMA queues and give tile-granular deps so
            # matmuls start as chunks land. w_up is split into u-halves,
            # loaded in the order the up-groups consume them. x block 0 is
            # loaded per-k-subtile (96 KB chunks) interleaved with the first
            # w_up half: subtile deps let up-chain step k start as soon as its
            # own x/w chunks land instead of waiting for whole tiles on a
            # single ~70 GB/s DMA queue.
            bu_s = const.tile([P, NU], dt_bias)
            nc.sync.dma_start(bu_s, bu.ap())

            NQ = 2 if U % (2 * P) == 0 else 1
            UQ = U // NQ
            wu_q = [[None] * KU for _ in range(NQ)]
            nc.scalar.dma_start(xbs[0][:, 0, :], xg3[:, 0, 0 : blocks[0]])
            for k in range(KU):
                wt = wpool.tile([P, UQ], dt, tag=f"wu0_{k}", name="wuq")
                nc.scalar.dma_start(wt, wu3[:, k, 0:UQ])
                wu_q[0][k] = wt
            for k in range(1, KU):
                nc.scalar.dma_start(xbs[0][:, k, :], xg3[:, k, 0 : blocks[0]])
            for q in range(1, NQ):
                for k in range(KU):
                    wt = wpool.tile([P, UQ], dt, tag=f"wu{q}_{k}", name="wuq")
                    nc.sync.dma_start(wt, wu3[:, k, q * UQ : (q + 1) * UQ])
                    wu_q[q][k] = wt

            def wu_slice(k, ut):
                u0 = ut * P
                q, r = divmod(u0, UQ)
                return wu_q[q][k][:, r : r + P]

            bd_s = const.tile([P, ND], dt_bias)

            # Both projections run k-outer over groups of up to 8 interleaved
            # PSUM accumulation chains (8 PSUM banks): each arriving weight
            # chunk unlocks GRP matmuls instead of 1, keeping the startup ramp
            # close to PE-bound instead of chunk-arrival-bound.
            GRP = min(8, NU, ND)
            csls = []
            c0 = 0
            for CB in blocks:
                csls.append(slice(c0, c0 + CB))
                c0 += CB

            wd_k = [None] * KD

            def up_phase(bi):
                CB = blocks[bi]
                h_tiles, act_insts = [], []
                for ug in range(0, NU, GRP):
                    pss = [
                        psum_pool.tile([P, CB], mybir.dt.float32, tag="ps", name="ps")
                        for _ in range(GRP)
                    ]
                    for k in range(KU):
                        for j in range(GRP):
                            nc.tensor.matmul(
                                pss[j],
                                wu_slice(k, ug + j),
                                xbs[bi][:, k, :],
                                start=(k == 0),
                                stop=(k == KU - 1),
                            )
                    for j in range(GRP):
                        hbt = hpool.tile([P, CB], dt, tag="h", name="hbt")
                        a = nc.scalar.activation(
                            hbt,
                            pss[j],
                            mybir.ActivationFunctionType.Gelu,
                            bias=bu_s[:, ug + j : ug + j + 1],
                            scale=1.0,
                        )
                        act_insts.append(a)
                        h_tiles.append(hbt)
                return h_tiles, act_insts

            def down_phase(bi, h_tiles, last):
                CB = blocks[bi]
                csl = csls[bi]
                # Final block uses half-size groups so the second group's
                # matmuls overlap the first group's evictions, shortening the
                # post-last-matmul tail.
                dgrp = GRP if not last else max(1, min(GRP, ND // 2))
                for dg in range(0, ND, dgrp):
                    nj = min(dgrp, ND - dg)
                    pss = [
                        psum_pool.tile([P, CB], mybir.dt.float32, tag="ps", name="ps")
                        for _ in range(nj)
                    ]
                    for k in range(KD):
                        for j in range(nj):
                            nc.tensor.matmul(
                                pss[j],
                                wd_k[k][:, (dg + j) * P : (dg + j + 1) * P],
                                h_tiles[k],
                                start=(k == 0),
                                stop=(k == KD - 1),
                            )
                    for j in range(nj):
                        yb = ypool.tile([P, CB], dt_out, tag="y", name="yb")
                        nc.vector.tensor_scalar_add(yb, pss[j], bd_s[:, dg + j : dg + j + 1])
                        nc.sync.dma_start(y3[:, dg + j, csl], yb)

            h0, acts0 = up_phase(0)

            # Everything not needed until block-0's down phase or later (the
            # other x blocks, b_down, all of w_down) is gated behind an
            # up-phase group-1 eviction so those transfers don't compete for
            # HBM bandwidth with the w_up chunks the ramp is waiting on.
            from concourse.tile_rust import add_dep_helper

            gate = acts0[0].ins

            def gated_dma(dst, src):
                di = nc.sync.dma_start(dst, src)
                add_dep_helper(di.ins, gate, sync=True, reason="defer until ramp done")

            for bi in range(1, len(blocks)):
                xbs[bi] = xpool.tile([P, KU, blocks[bi]], dt, tag=f"x{bi}", name=f"xb{bi}")
                gated_dma(xbs[bi], xg3[:, :, csls[bi]])
            gated_dma(bd_s, bd.ap())
            for k in range(KD):
                wt = wpool.tile([P, D], dt, tag=f"wd{k}", name="wdk")
                gated_dma(wt, wd3[:, k, :])
                wd_k[k] = wt

            down_phase(0, h0, last=(len(blocks) == 1))
            for bi in range(1, len(blocks)):
                hb, _ = up_phase(bi)
                down_phase(bi, hb, last=(bi == len(blocks) - 1))

    nc.compile()
    return nc


def _route(xf, w_router):
    """Host-side routing: top-2 expert ids + softmax weights per token."""
    logits = xf @ w_router.T  # [T, E]
    order = np.argsort(-logits, axis=1, kind="stable")[:, :TOPK]  # [T, 2]
    top = np.take_along_axis(logits, order, axis=1)
    m = top.max(axis=1, keepdims=True)
    ex = np.exp(top - m)
    rw = ex / ex.sum(axis=1, keepdims=True)  # [T, 2]
    return order, rw


def kernel(**inputs):
    global _last_results
    from concourse.bass_utils import run_bass_kernel_spmd

    x = np.ascontiguousarray(np.asarray(inputs["x"]), dtype=np.float32)
    w_router = np.asarray(inputs["w_router"]).astype(np.float32, copy=False)
    w_up = np.asarray(inputs["w_up"]).astype(np.float32, copy=False)
    b_up = np.asarray(inputs["b_up"]).astype(np.float32, copy=False)
    w_down = np.asarray(inputs["w_down"]).astype(np.float32, copy=False)
    b_down = np.asarray(inputs["b_down"]).astype(np.float32, copy=False)

    Bx, Sx, Dx = x.shape
    Tx = Bx * Sx
    xf = x.reshape(Tx, Dx)

    order, rw = _route(xf, w_router)

    idx_list, wgt_list = [], []
    for e in range(E):
        rows, slots = np.nonzero(order == e)
        idx_list.append(rows.astype(np.int64))
        wgt_list.append(rw[rows, slots].astype(np.float32))

    maxc = max(len(ii) for ii in idx_list)
    C = max(256, -(-maxc // 64) * 64)

    cache_key = (C, _mm_dtype_name())
    if cache_key not in _prog_cache:
        _prog_cache[cache_key] = _build_program(C)
    nc = _prog_cache[cache_key]

    if _mm_dtype_name() == "bf16":
        import ml_dtypes

        mm_np = ml_dtypes.bfloat16
    elif _mm_dtype_name() == "fp16":
        mm_np = np.float16
    else:
        mm_np = np.float32

    in_maps = []
    for e in range(E):
        idx = idx_list[e]
        xg = np.zeros((C, Dx), np.float32)
        xg[: len(idx)] = xf[idx]
        in_maps.append(
            {
                "xgT": np.ascontiguousarray(xg.T).astype(mm_np, copy=False),
                "wuT": np.ascontiguousarray(w_up[e].T).astype(mm_np, copy=False),
                "wdT": np.ascontiguousarray(w_down[e].T).astype(mm_np, copy=False),
                "bu": np.ascontiguousarray(b_up[e].reshape(U // P, P).T),
                "bd": np.ascontiguousarray(b_down[e].reshape(D // P, P).T),
            }
        )

    t0 = time.perf_counter()
    res = run_bass_kernel_spmd(nc, in_maps, core_ids=list(range(E)))
    t1 = time.perf_counter()
    _last_results = res
    if os.environ.get("KERNEL_VERBOSE"):
        print(f"[kernel] device run wall time: {(t1 - t0) * 1e3:.1f} ms")

    out = np.zeros((Tx, Dx), np.float32)
    for e in range(E):
        idx = idx_list[e]
        y = res.results[e]["yT"].T  # [C, D]
        out[idx] += wgt_list[e][:, None] * y[: len(idx)]

    return out.reshape(Bx, Sx, Dx)


# revision 36
# speedup vs baseline: 1.0307x; 1.0307x over previous
# MoE block (top-2 of 8 experts) on 8 trn2 NeuronCores, expert-parallel.
#
# Sharding strategy:
#   - Core e owns expert e's weights (expert-parallel: each weight byte is read
#     from HBM exactly once across the fleet).
#   - Routing (x @ w_router.T, top-2, softmax) + token dispatch are computed on
#     the host as part of input sharding; core e receives the (transposed,
#     padded) batch of tokens routed to expert e.
#   - Device kernel per core: h.T = gelu(w_up @ x_g.T + b_up);
#     y.T = w_down @ h.T + b_down  — features on partitions, tokens on the
#     matmul free dimension, so every DMA is contiguous (no on-device
#     transposes needed).
#   - Unshard: host scatter-adds the per-expert outputs weighted by the top-2
#     softmax router weights.
import os
import time

import numpy as np

B, S, D, U, E, TOPK = 2, 2048, 1024, 4096, 8, 2
T = B * S
P = 128

_last_results = None  # BassKernelResults of the most recent device run (for test.py)
_prog_cache = {}


def _split_blocks(C):
    """Split C token columns into blocks of <=512 (PSUM bank / fp32 matmul
    free-dim limit), as equal as possible in multiples of 128."""
    nb = -(-C // 512)
    base = C // (128 * nb) * 128
    blocks = [base] * nb
    rem = C - base * nb
    i = 0
    while rem > 0:
        add = min(128, rem)
        blocks[i % nb] += add
        rem -= add
        i += 1
    assert sum(blocks) == C and all(b <= 512 for b in blocks)
    return blocks


def _mm_dtype_name():
    # fp16: same PE rate as bf16 (1 cyc/row) but 11-bit mantissa -> ~4e-4
    # relative error vs the fp32 reference (values here are far inside fp16
    # range). Measured: fp32 1017us/1.8e-6, fp32r 458us/2.1e-4,
    # bf16 357us/3.3e-3, fp16 346us/4.1e-4.
    return os.environ.get("KERNEL_MM_DTYPE", "fp16")


def _build_program(C):
    import concourse.bacc as bacc
    import concourse.mybir as mybir
    import concourse.tile as tile

    # Matmul operand dtype (measured issue rates on TRN2): fp32 ~4 cyc/row
    # (lowered to 2 half-rate passes), fp32r ~1.5, bf16/fp16 ~1 cyc/row.
    # PSUM accumulation is fp32 throughout.
    dt = {
        "fp32": mybir.dt.float32,
        "fp32r": mybir.dt.float32r,
        "bf16": mybir.dt.bfloat16,
        "fp16": mybir.dt.float16,
    }[_mm_dtype_name()]
    dt_bias = mybir.dt.float32
    dt_out = mybir.dt.float32
    KU = D // P  # 8   k-subtiles for the up-projection (contract over D)
    NU = U // P  # 32  output tiles of the up-projection
    KD = U // P  # 32  k-subtiles for the down-projection (contract over U)
    ND = D // P  # 8   output tiles of the down-projection

    nc = bacc.Bacc("TRN2", target_bir_lowering=False, debug=False, num_devices=E)

    xgT = nc.dram_tensor("xgT", [D, C], dt, kind="ExternalInput")  # gathered x, transposed
    wuT = nc.dram_tensor("wuT", [D, U], dt, kind="ExternalInput")  # w_up[e].T
    wdT = nc.dram_tensor("wdT", [U, D], dt, kind="ExternalInput")  # w_down[e].T
    bu = nc.dram_tensor("bu", [P, NU], dt_bias, kind="ExternalInput")  # b_up[e] as [128, 32]
    bd = nc.dram_tensor("bd", [P, ND], dt_bias, kind="ExternalInput")  # b_down[e] as [128, 8]
    yT = nc.dram_tensor("yT", [D, C], dt_out, kind="ExternalOutput")

    xg3 = xgT.ap().rearrange("(ko p) c -> p ko c", p=P)  # [128, 8, C]
    wu3 = wuT.ap().rearrange("(ko p) u -> p ko u", p=P)  # [128, 8, U]
    wd3 = wdT.ap().rearrange("(ko p) d -> p ko d", p=P)  # [128, 32, D]
    y3 = yT.ap().rearrange("(ko p) c -> p ko c", p=P)  # [128, 8, C]

    with tile.TileContext(nc) as tc:
        with (
            tc.tile_pool(name="const", bufs=1) as const,
            tc.tile_pool(name="weights", bufs=1) as wpool,
            tc.tile_pool(name="xpool", bufs=1) as xpool,
            tc.tile_pool(name="hpool", bufs=NU + 3) as hpool,
            tc.tile_pool(name="ypool", bufs=3) as ypool,
            tc.tile_pool(name="psum", bufs=8, space="PSUM") as psum_pool,
        ):
            blocks = _split_blocks(C)

            # DMA emission order tracks first-use order: x block 0, then w_up
            # (first up-chains), then remaining x blocks, biases, w_down.
            # Early transfers are enqueued on BOTH HWDGE-capable engines
            # (Scalar + Sync, ~0.7us per enqueue each) so enqueue
            # serialization doesn't pace the startup ramp. Scalar's 9
            # enqueues finish ~14us, well before its first gelu (~20us).
            xbs = [None] * len(blocks)
            xbs[0] = xpool.tile([P, KU, blocks[0]], dt, tag="x0", name="xb0")

            # Both weight matrices are SBUF-resident (16.8 MB in fp16): each
            # is loaded exactly once, as per-k-subtile fully-contiguous DMAs
            # that spread across the DMA queues and give tile-granular deps so
            # matmuls start as chunks land. w_up is split into u-halves,
            # loaded in the order the up-groups consume them. The startup ramp
            # is near the aggregate-HBM bound (~300 GB/s for ~10 MB of x+w_up),
            # so keep the early enqueue count minimal.
            bu_s = const.tile([P, NU], dt_bias)
            nc.sync.dma_start(bu_s, bu.ap())

            NQ = 2 if U % (2 * P) == 0 else 1
            UQ = U // NQ
            wu_q = [[None] * KU for _ in range(NQ)]
            nc.scalar.dma_start(xbs[0], xg3[:, :, 0 : blocks[0]])
            for k in range(KU):
                wt = wpool.tile([P, UQ], dt, tag=f"wu0_{k}", name="wuq")
                nc.scalar.dma_start(wt, wu3[:, k, 0:UQ])
                wu_q[0][k] = wt
            for q in range(1, NQ):
                for k in range(KU):
                    wt = wpool.tile([P, UQ], dt, tag=f"wu{q}_{k}", name="wuq")
                    nc.sync.dma_start(wt, wu3[:, k, q * UQ : (q + 1) * UQ])
                    wu_q[q][k] = wt

            def wu_slice(k, ut):
                u0 = ut * P
                q, r = divmod(u0, UQ)
                return wu_q[q][k][:, r : r + P]

            bd_s = const.tile([P, ND], dt_bias)

            # Both projections run k-outer over groups of up to 8 interleaved
            # PSUM accumulation chains (8 PSUM banks): each arriving weight
            # chunk unlocks GRP matmuls instead of 1, keeping the startup ramp
            # close to PE-bound instead of chunk-arrival-bound.
            GRP = min(8, NU, ND)
            csls = []
            c0 = 0
            for CB in blocks:
                csls.append(slice(c0, c0 + CB))
                c0 += CB

            wd_k = [None] * KD

            def up_phase(bi):
                CB = blocks[bi]
                h_tiles, act_insts = [], []
                for ug in range(0, NU, GRP):
                    pss = [
                        psum_pool.tile([P, CB], mybir.dt.float32, tag="ps", name="ps")
                        for _ in range(GRP)
                    ]
                    for k in range(KU):
                        for j in range(GRP):
                            nc.tensor.matmul(
                                pss[j],
                                wu_slice(k, ug + j),
                                xbs[bi][:, k, :],
                                start=(k == 0),
                                stop=(k == KU - 1),
                            )
                    for j in range(GRP):
                        hbt = hpool.tile([P, CB], dt, tag="h", name="hbt")
                        a = nc.scalar.activation(
                            hbt,
                            pss[j],
                            mybir.ActivationFunctionType.Gelu,
                            bias=bu_s[:, ug + j : ug + j + 1],
                            scale=1.0,
                        )
                        act_insts.append(a)
                        h_tiles.append(hbt)
                return h_tiles, act_insts

            def down_phase(bi, h_tiles, last):
                CB = blocks[bi]
                csl = csls[bi]
                # Final block uses half-size groups so the second group's
                # matmuls overlap the first group's evictions, shortening the
                # post-last-matmul tail.
                dgrp = GRP if not last else max(1, min(GRP, ND // 2))
                for dg in range(0, ND, dgrp):
                    nj = min(dgrp, ND - dg)
                    pss = [
                        psum_pool.tile([P, CB], mybir.dt.float32, tag="ps", name="ps")
                        for _ in range(nj)
                    ]
                    for k in range(KD):
                        for j in range(nj):
                            nc.tensor.matmul(
                                pss[j],
                                wd_k[k][:, (dg + j) * P : (dg + j + 1) * P],
                                h_tiles[k],
                                start=(k == 0),
                                stop=(k == KD - 1),
                            )
                    for j in range(nj):
                        yb = ypool.tile([P, CB], dt_out, tag="y", name="yb")
                        nc.vector.tensor_scalar_add(yb, pss[j], bd_s[:, dg + j : dg + j + 1])
                        nc.sync.dma_start(y3[:, dg + j, csl], yb)

            h0, acts0 = up_phase(0)

            # Everything not needed until block-0's down phase or later (the
            # other x blocks, b_down, all of w_down) is gated behind an
            # up-phase group-1 eviction so those transfers don't compete for
            # HBM bandwidth with the w_up chunks the ramp is waiting on.
            from concourse.tile_rust import add_dep_helper

            gate = acts0[0].ins

            def gated_dma(dst, src):
                di = nc.sync.dma_start(dst, src)
                add_dep_helper(di.ins, gate, sync=True, reason="defer until ramp done")

            for bi in range(1, len(blocks)):
                xbs[bi] = xpool.tile([P, KU, blocks[bi]], dt, tag=f"x{bi}", name=f"xb{bi}")
                gated_dma(xbs[bi], xg3[:, :, csls[bi]])
            gated_dma(bd_s, bd.ap())
            for k in range(KD):
                wt = wpool.tile([P, D], dt, tag=f"wd{k}", name="wdk")
                gated_dma(wt, wd3[:, k, :])
                wd_k[k] = wt

            down_phase(0, h0, last=(len(blocks) == 1))
            for bi in range(1, len(blocks)):
                hb, _ = up_phase(bi)
                down_phase(bi, hb, last=(bi == len(blocks) - 1))

    nc.compile()
    return nc


def _route(xf, w_router):
    """Host-side routing: top-2 expert ids + softmax weights per token."""
    logits = xf @ w_router.T  # [T, E]
    order = np.argsort(-logits, axis=1, kind="stable")[:, :TOPK]  # [T, 2]
    top = np.take_along_axis(logits, order, axis=1)
    m = top.max(axis=1, keepdims=True)
    ex = np.exp(top - m)
    rw = ex / ex.sum(axis=1, keepdims=True)  # [T, 2]
    return order, rw


def kernel(**inputs):
    global _last_results
    from concourse.bass_utils import run_bass_kernel_spmd

    x = np.ascontiguousarray(np.asarray(inputs["x"]), dtype=np.float32)
    w_router = np.asarray(inputs["w_router"]).astype(np.float32, copy=False)
    w_up = np.asarray(inputs["w_up"]).astype(np.float32, copy=False)
    b_up = np.asarray(inputs["b_up"]).astype(np.float32, copy=False)
    w_down = np.asarray(inputs["w_down"]).astype(np.float32, copy=False)
    b_down = np.asarray(inputs["b_down"]).astype(np.float32, copy=False)

    Bx, Sx, Dx = x.shape
    Tx = Bx * Sx
    xf = x.reshape(Tx, Dx)

    order, rw = _route(xf, w_router)

    idx_list, wgt_list = [], []
    for e in range(E):
        rows, slots = np.nonzero(order == e)
        idx_list.append(rows.astype(np.int64))
        wgt_list.append(rw[rows, slots].astype(np.float32))

    maxc = max(len(ii) for ii in idx_list)
    C = max(256, -(-maxc // 16) * 16)

    cache_key = (C, _mm_dtype_name())
    if cache_key not in _prog_cache:
        _prog_cache[cache_key] = _build_program(C)
    nc = _prog_cache[cache_key]

    if _mm_dtype_name() == "bf16":
        import ml_dtypes

        mm_np = ml_dtypes.bfloat16
    elif _mm_dtype_name() == "fp16":
        mm_np = np.float16
    else:
        mm_np = np.float32

    in_maps = []
    for e in range(E):
        idx = idx_list[e]
        xg = np.zeros((C, Dx), np.float32)
        xg[: len(idx)] = xf[idx]
        in_maps.append(
            {
                "xgT": np.ascontiguousarray(xg.T).astype(mm_np, copy=False),
                "wuT": np.ascontiguousarray(w_up[e].T).astype(mm_np, copy=False),
                "wdT": np.ascontiguousarray(w_down[e].T).astype(mm_np, copy=False),
                "bu": np.ascontiguousarray(b_up[e].reshape(U // P, P).T),
                "bd": np.ascontiguousarray(b_down[e].reshape(D // P, P).T),
            }
        )

    t0 = time.perf_counter()
    res = run_bass_kernel_spmd(nc, in_maps, core_ids=list(range(E)))
    t1 = time.perf_counter()
    _last_results = res
    if os.environ.get("KERNEL_VERBOSE"):
        print(f"[kernel] device run wall time: {(t1 - t0) * 1e3:.1f} ms")

    out = np.zeros((Tx, Dx), np.float32)
    for e in range(E):
        idx = idx_list[e]
        y = res.results[e]["yT"].T  # [C, D]
        out[idx] += wgt_list[e][:, None] * y[: len(idx)]

    return out.reshape(Bx, Sx, Dx)
